# revision 5
# baseline (speedup 1.0000x reference)
"""Trainium2 Bass kernel: per-sample modulated/demodulated 3x3 conv via
Winograd F(2x2, 3x3).

Problem: x (8,512,32,32), s (8,512), w (512,512,3,3) ->
  wm[b,o,i,ky,kx] = w * (s[b,i]+1); demod by rsqrt(sum wm^2 + eps) per (b,o);
  y[b] = conv2d_same(x[b], wm[b]).

Sharding: data-parallel over batch, 1 sample per NeuronCore (8 cores).

Work split (everything scalar-foldable is off-device):
  host pre:  x' = x*(1+s) padded, col-deinterleaved, bf16.
             w'' = (G @ w) * den[b,o]: the vertical half of the Winograd
             weight transform (12 planes, middle column pre-halved) --
             smaller than full U (12 vs 16 planes of HBM traffic); the
             horizontal combine is 12 cheap DVE ops on device.
  device:    V = B^T x' B (DVE scalar_tensor_tensor ops: the only
             InstTensor* with 4x DVE perf mode support);
             U nu-combine: t = k0+k2; u_nu1/2 = t/2 +- k1h;
             M = V^T-stationary matmuls: lhsT = V tile-slices [128cin, 128
             tiles], moving = U rows [128cin, 512 cout] -> 128 matmuls of
             512 cols (half the LDWEIGHTS of the U-stationary version);
             PSUM group (xi,nu,t2) = [128, 512] f32 (1 bank, 8 in flight);
             drains PSUM->bf16 split across ACT and DVE; DMA M out.
  host post: Y = A^T M A  (tiny; vectorized numpy during unshard).

Measured rates (this chip): DVE tensor_tensor ~0.82 ns/elem bf16, ACT
~1.3 ns/elem, gpsimd ~3.5 ns/elem AND it contends with DVE for SBUF ports
-> gpsimd only issues the output DMAs. PE group order nu = [0,3,1,2] so
matmuls start while the k1h planes and nu-combines are still in flight.
"""

import sys

if "/opt/trn_rl_repo" not in sys.path:
    sys.path.insert(0, "/opt/trn_rl_repo")

import numpy as np

B = 8
CIN = 512
COUT = 512
H = 32
W = 32
NCH = CIN // 128  # cin chunks
EPS = 1e-8

_compiled_nc = None


def _build():
    import concourse.tile as tile
    from concourse import bacc, mybir

    F32 = mybir.dt.float32
    BF16 = mybir.dt.bfloat16
    ALU = mybir.AluOpType

    nc = bacc.Bacc("TRN2", target_bir_lowering=False, debug=False, num_devices=B)
    # x' padded + col-deinterleaved: [cin, 34 rows, 2 (even/odd), 17]
    xm_d = nc.dram_tensor("xm", [CIN, 34, 2, 17], BF16, kind="ExternalInput").ap()
    # w'' = (G w)*den packed [k, cin_chunk, 128, xi, cout]; k=1 pre-halved
    w2_d = nc.dram_tensor("w2", [3, NCH, 128, 4, COUT], BF16, kind="ExternalInput").ap()
    # Winograd-domain output: [xi, nu, 128 tiles, t2, cout]
    m_d = nc.dram_tensor("m", [4, 4, 128, 2, COUT], BF16, kind="ExternalOutput").ap()

    with tile.TileContext(nc) as tc:
        with (
            tc.tile_pool(name="xpool", bufs=1) as xpool,
            tc.tile_pool(name="epool", bufs=1) as epool,
            tc.tile_pool(name="vpool", bufs=1) as vpool,
            tc.tile_pool(name="wpool", bufs=1) as wpool,
            tc.tile_pool(name="dpool", bufs=1) as dpool,
            tc.tile_pool(name="misc", bufs=1) as misc,
            tc.tile_pool(name="psum", bufs=8, space="PSUM") as psum,
        ):
            xp = [
                xpool.tile([128, 34, 2, 17], BF16, name=f"xp{c}", tag=f"x{c}")
                for c in range(NCH)
            ]
            # E = vertical pass: [xi, ty, colhalf, 17]
            ev = [
                epool.tile([128, 4, 16, 2, 17], BF16, name=f"e{c}", tag=f"e{c}")
                for c in range(NCH)
            ]
            # V: [xi, nu, ty, tx]
            vt = [
                vpool.tile([128, 4, 4, 16, 16], BF16, name=f"v{c}", tag=f"v{c}")
                for c in range(NCH)
            ]
            # w'' raw planes: [k, xi, cout]
            w2 = [
                wpool.tile([128, 3, 4, COUT], BF16, name=f"w2_{c}", tag=f"w2_{c}")
                for c in range(NCH)
            ]
            # computed U planes nu=1,2: [2, xi, cout]
            uu = [
                wpool.tile([128, 2, 4, COUT], BF16, name=f"uu{c}", tag=f"uu{c}")
                for c in range(NCH)
            ]
            tsc = [
                wpool.tile([128, 4, COUT], BF16, name=f"ts{c}", tag=f"ts{c}")
                for c in range(NCH)
            ]
            # drained M planes per (xi,nu): [t2, cout]
            dr = [
                dpool.tile([128, 2, COUT], BF16, name=f"d{i}", tag=f"d{i}")
                for i in range(16)
            ]
            junk = misc.tile([128, 512], BF16, name="junk", tag="junk")

            # --- PE warmup while first DMAs are in flight (HAM clock ramp)
            nc.gpsimd.memset(junk, 0.0)
            warm = psum.tile([128, 512], F32, name="warm", tag="acc")
            for _ in range(10):
                nc.tensor.matmul(
                    warm, lhsT=junk[:, 0:128], rhs=junk, start=True, stop=True
                )

            # --- input DMAs: x' chunks first, then w'' k-planes in
            # consumption order (k0 -> k2 -> k1h).
            for c in range(NCH):
                nc.sync.dma_start(out=xp[c], in_=xm_d[c * 128 : (c + 1) * 128])
            for k in (0, 2, 1):
                for c in range(NCH):
                    nc.sync.dma_start(
                        out=w2[c][:, k], in_=w2_d[k, c]
                    )

            def stt(eng, out, in0, scalar, op0, in1, op1):
                eng.scalar_tensor_tensor(
                    out=out, in0=in0, scalar=scalar, in1=in1, op0=op0, op1=op1
                )

            # --- forward transform (all DVE, scalar_tensor_tensor for the
            # 4x perf mode). Vertical per chunk, then nu=0 horizontal so the
            # PE can start; nu=3 next (matches k2 arrival), then U-combines,
            # then nu=1,2.
            def vpass(c):
                xr = xp[c].rearrange("p (r t) h s -> p r t (h s)", t=2)
                er = ev[c].rearrange("p x a h s -> p x a (h s)")
                a, su, m = ALU.add, ALU.subtract, ALU.mult
                v = nc.vector
                stt(v, er[:, 0], xr[:, 0:16, 0], 1.0, m, xr[:, 1:17, 0], su)
                stt(v, er[:, 1], xr[:, 0:16, 1], 1.0, m, xr[:, 1:17, 0], a)
                stt(v, er[:, 2], xr[:, 1:17, 0], 1.0, m, xr[:, 0:16, 1], su)
                stt(v, er[:, 3], xr[:, 0:16, 1], 1.0, m, xr[:, 1:17, 1], su)

            def hpass(c, nu):
                ee = ev[c][:, :, :, 0, :]  # [128, 4, 16, 17] even cols
                eo = ev[c][:, :, :, 1, :]  # odd cols
                out = vt[c][:, :, nu]  # [128, 4, 16, 16]
                a, su, m = ALU.add, ALU.subtract, ALU.mult
                v = nc.vector
                if nu == 0:
                    stt(v, out, ee[:, :, :, 0:16], 1.0, m, ee[:, :, :, 1:17], su)
                elif nu == 1:
                    stt(v, out, eo[:, :, :, 0:16], 1.0, m, ee[:, :, :, 1:17], a)
                elif nu == 2:
                    stt(v, out, ee[:, :, :, 1:17], 1.0, m, eo[:, :, :, 0:16], su)
                else:
                    stt(v, out, eo[:, :, :, 0:16], 1.0, m, eo[:, :, :, 1:17], su)

            for c in range(NCH):
                vpass(c)
                hpass(c, 0)
            for c in range(NCH):
                hpass(c, 3)
            # U nu-combine: t = k0 + k2; u1 = t/2 + k1h; u2 = t/2 - k1h
            for c in range(NCH):
                a, su, m = ALU.add, ALU.subtract, ALU.mult
                v = nc.vector
                stt(v, tsc[c], w2[c][:, 0], 1.0, m, w2[c][:, 2], a)
                stt(v, uu[c][:, 0], tsc[c], 0.5, m, w2[c][:, 1], a)
                stt(v, uu[c][:, 1], tsc[c], 0.5, m, w2[c][:, 1], su)
            for c in range(NCH):
                hpass(c, 1)
            for c in range(NCH):
                hpass(c, 2)

            def umov(c, nu, xi):
                if nu == 0:
                    return w2[c][:, 0, xi, :]
                if nu == 3:
                    return w2[c][:, 2, xi, :]
                return uu[c][:, nu - 1, xi, :]

            # --- matmuls + drains + M stores, pipelined per (nu, xi, t2).
            gi = 0
            for nu in (0, 3, 1, 2):
                for xi in range(4):
                    for t2 in range(2):
                        g = psum.tile(
                            [128, COUT], F32, name=f"acc{nu}_{xi}_{t2}", tag="acc"
                        )
                        for c in range(NCH):
                            nc.tensor.matmul(
                                g,
                                lhsT=vt[c][:, xi, nu, 8 * t2 : 8 * t2 + 8, :],
                                rhs=umov(c, nu, xi),
                                start=(c == 0),
                                stop=(c == NCH - 1),
                            )
                        # drain to SBUF bf16; split ACT (2/3) and DVE (1/3)
                        di = 4 * xi + nu
                        if gi % 3 == 2:
                            nc.vector.tensor_copy(dr[di][:, t2], g)
                        else:
                            nc.scalar.copy(dr[di][:, t2], g)
                        gi += 1
                        if t2 == 1:
                            nc.gpsimd.dma_start(out=m_d[xi, nu], in_=dr[di])

    nc.compile()
    return nc


_G = np.array([[1, 0, 0], [0.5, 0.5, 0.5], [0.5, -0.5, 0.5], [0, 0, 1]], np.float64)
_AT = np.array([[1, 1, 1, 0], [0, 1, -1, -1]], np.float32)


def _prep_in_maps(x, s, w):
    """Host-side fold + pack: returns per-core input dicts."""
    import ml_dtypes

    x = np.asarray(x, np.float64)
    s = np.asarray(s, np.float64)
    w = np.asarray(w, np.float64)
    s1 = s + 1.0  # (b, cin)
    wsq = (w * w).sum(axis=(2, 3))  # (cout, cin)
    den = 1.0 / np.sqrt((s1 * s1) @ wsq.T + EPS)  # (b, cout)

    # w''[o,i,xi,kx] = sum_ky G[xi,ky] w[o,i,ky,kx]; pre-halve kx=1 plane
    W2 = np.einsum("xk,oikl->oixl", _G, w, optimize=True)
    W2[:, :, :, 1] *= 0.5

    # x': modulate, pad to 34x34, deinterleave columns
    xm = x * s1[:, :, None, None]  # (b, cin, 32, 32)
    xp = np.zeros((B, CIN, 34, 34), np.float32)
    xp[:, :, 1:33, 1:33] = xm
    xpk = np.empty((B, CIN, 34, 2, 17), np.float32)
    xpk[:, :, :, 0, :] = xp[:, :, :, 0::2]
    xpk[:, :, :, 1, :] = xp[:, :, :, 1::2]
    xpk = xpk.astype(ml_dtypes.bfloat16)

    in_maps = []
    for b in range(B):
        wb = W2 * den[b][:, None, None, None]  # (o, i, xi, k)
        # pack -> [k, c, p, xi, o]
        wp = wb.transpose(3, 1, 2, 0)  # (k, i, xi, o)
        wp = np.ascontiguousarray(wp.reshape(3, NCH, 128, 4, COUT)).astype(
            ml_dtypes.bfloat16
        )
        in_maps.append({"xm": np.ascontiguousarray(xpk[b]), "w2": wp})
    return in_maps


def _finish(res):
    """Host-side inverse transform: Y = A^T M A, assemble (b, cout, 32, 32)."""
    # m: (xi, nu, 128, t2, cout) bf16; tile index = t2*128+p, ty = tile//16
    M = np.stack([np.asarray(r["m"]).astype(np.float32) for r in res], axis=0)
    M = M.transpose(0, 1, 2, 4, 3, 5).reshape(B, 4, 4, 16, 16, COUT)
    # (b, xi, nu, ty, tx, co) -> y[b, co, 2ty+i, 2tx+j]
    y = np.einsum("ix,jn,bxntuo->botiuj", _AT, _AT, M, optimize=True)
    return np.ascontiguousarray(y.reshape(B, COUT, H, W))


def kernel(x, s, w):
    from concourse.bass_utils import run_bass_kernel_spmd

    global _compiled_nc
    if _compiled_nc is None:
        _compiled_nc = _build()
    nc = _compiled_nc

    in_maps = _prep_in_maps(x, s, w)
    res = run_bass_kernel_spmd(nc, in_maps, list(range(B))).results
    return _finish([res[i] for i in range(B)])
